# revision 12
# baseline (speedup 1.0000x reference)
"""AttnBlock1d Trainium2 kernel: 8-core SPMD, zero-collective sharding.

Sharding: core i handles (batch b = i//2, N-half = i%2). The input x[b] is
host-rolled along N so every core's query half sits at columns 0:1024 —
groupnorm stats, k/v (pointwise in N) and softmax are permutation-invariant
along N, so rolling commutes with everything except the q slice, which is
exactly the point.

Per-core pipeline (N=2048 keys, NQ=1024 queries, C=512, H=8 heads):
  groupnorm(x) -> h_ (bf16)
  q = qw@h_[:, :1024]+qb, k = kw@h_+kb (bf16)
  vT[n, c] = h_^T @ v_w^T (bf16, per-head 65-col blocks with a ones column)
  per head: scoresT[nk, nq] = k_h^T q_h (PSUM f32, head pairs row-packed on
  the PE array), exp via ScalarE (scale folded), out_u = vT_aug^T @ exp
  (M=65: row 64 accumulates the softmax denominator D), divide by D
  (DVE reciprocal + DMA partition-broadcast), + v_b
  proj + proj_b + residual x -> out[512, 1024]
"""

import os
import sys

import numpy as np

if "/opt/trn_rl_repo" not in sys.path:
    sys.path.insert(0, "/opt/trn_rl_repo")

import ml_dtypes

import concourse.bacc as bacc
import concourse.tile as tile
from concourse import mybir
from concourse.bass_utils import run_bass_kernel_spmd

F32 = mybir.dt.float32
BF16 = mybir.dt.bfloat16
AF = mybir.ActivationFunctionType
ALU = mybir.AluOpType

C = 512
N = 2048
NQ = 1024
H = 8
HC = 64
G = 32
EPS = 1e-6
SCALE = 1.0 / np.sqrt(C)

TRACE = False
LAST_RESULT = None


DEBUG_TAPS = False


def build_bacc():
    nc = bacc.Bacc()
    dbg = {}
    if DEBUG_TAPS:
        dbg["h0"] = nc.declare_dram_parameter("dbg_h0", [128, N], BF16, isOutput=True)
        dbg["mvg"] = nc.declare_dram_parameter("dbg_mvg", [G, 2], F32, isOutput=True)
        dbg["varg"] = nc.declare_dram_parameter("dbg_varg", [G, 1], F32, isOutput=True)
        dbg["q0"] = nc.declare_dram_parameter("dbg_q0", [128, NQ], BF16, isOutput=True)
        dbg["k0"] = nc.declare_dram_parameter("dbg_k0", [128, N], BF16, isOutput=True)
        dbg["vt0"] = nc.declare_dram_parameter("dbg_vt0", [128, H * 65], BF16, isOutput=True)
        dbg["exp0"] = nc.declare_dram_parameter("dbg_exp0", [128, N], BF16, isOutput=True)
        dbg["op0"] = nc.declare_dram_parameter("dbg_op0", [65, 512], F32, isOutput=True)
        dbg["s2"] = nc.declare_dram_parameter("dbg_s2", [128, 2], F32, isOutput=True)
        dbg["gps"] = nc.declare_dram_parameter("dbg_gps", [G, 2], F32, isOutput=True)
        dbg["attn0"] = nc.declare_dram_parameter("dbg_attn0", [128, NQ], BF16, isOutput=True)

    x_d = nc.declare_dram_parameter("x", [C, N], F32, isOutput=False)
    qwt_d = nc.declare_dram_parameter("qwt", [C, C], BF16, isOutput=False)
    kwt_d = nc.declare_dram_parameter("kwt", [C, C], BF16, isOutput=False)
    vwt_d = nc.declare_dram_parameter("vwt", [C, C], BF16, isOutput=False)
    pwt_d = nc.declare_dram_parameter("pwt", [C, C], BF16, isOutput=False)
    qb_d = nc.declare_dram_parameter("qb", [C, 1], F32, isOutput=False)
    kb_d = nc.declare_dram_parameter("kb", [C, 1], F32, isOutput=False)
    pb_d = nc.declare_dram_parameter("pb", [C, 1], F32, isOutput=False)
    vbh_d = nc.declare_dram_parameter("vbh", [HC, H], F32, isOutput=False)
    gam_d = nc.declare_dram_parameter("gam", [C, 1], F32, isOutput=False)
    bet_d = nc.declare_dram_parameter("bet", [C, 1], F32, isOutput=False)
    gmap_d = nc.declare_dram_parameter("gmap", [C, G], F32, isOutput=False)
    gmapt_d = nc.declare_dram_parameter("gmapt", [G, C], F32, isOutput=False)
    out_d = nc.declare_dram_parameter("out", [C, NQ], F32, isOutput=True)

    from contextlib import ExitStack

    with tile.TileContext(nc) as tc, ExitStack() as es:
        const = es.enter_context(tc.tile_pool(name="const", bufs=1))
        data = es.enter_context(tc.tile_pool(name="data", bufs=1))
        work = es.enter_context(tc.tile_pool(name="work", bufs=2))
        expp = es.enter_context(tc.tile_pool(name="expp", bufs=12))
        osbp = es.enter_context(tc.tile_pool(name="osbp", bufs=3))
        psA = es.enter_context(tc.tile_pool(name="psA", bufs=4, space="PSUM"))
        psB = es.enter_context(tc.tile_pool(name="psB", bufs=1, space="PSUM"))
        dpool = es.enter_context(tc.tile_pool(name="dpool", bufs=2, space="DRAM"))

        # ---- constant loads ----
        def load4(dram, shape, dt, tagp):
            ts = []
            for t in range(4):
                s = const.tile(shape, dt, tag=f"{tagp}{t}")
                nc.sync.dma_start(out=s, in_=dram[t * 128:(t + 1) * 128, :])
                ts.append(s)
            return ts

        qwt = load4(qwt_d, [128, C], BF16, "qwt")
        kwt = load4(kwt_d, [128, C], BF16, "kwt")
        vwt = load4(vwt_d, [128, C], BF16, "vwt")
        pwt = load4(pwt_d, [128, C], BF16, "pwt")
        qb = load4(qb_d, [128, 1], F32, "qb")
        kb = load4(kb_d, [128, 1], F32, "kb")
        pb = load4(pb_d, [128, 1], F32, "pb")
        gam = load4(gam_d, [128, 1], F32, "gam")
        bet = load4(bet_d, [128, 1], F32, "bet")
        gmap = load4(gmap_d, [128, G], F32, "gmap")
        gmapt = const.tile([G, C], F32, tag="gmapt")
        nc.sync.dma_start(out=gmapt, in_=gmapt_d[:, :])
        vbh = const.tile([HC, H], F32, tag="vbh")
        nc.sync.dma_start(out=vbh, in_=vbh_d[:, :])
        eps32 = const.tile([G, 1], F32, tag="eps32")
        nc.vector.memset(eps32, EPS)

        xs = []
        for t in range(4):
            xt = data.tile([128, N], F32, tag=f"x{t}")
            nc.sync.dma_start(out=xt, in_=x_d[t * 128:(t + 1) * 128, :])
            xs.append(xt)

        # ---- groupnorm stats ----
        stats2s = []
        for t in range(4):
            st = work.tile([128, 4, 6], F32, tag="bnst")
            for sg in range(4):
                nc.vector.bn_stats(out=st[:, sg, :], in_=xs[t][:, sg * 512:(sg + 1) * 512])
            mv = work.tile([128, 2], F32, tag="bnmv")
            nc.vector.bn_aggr(out=mv, in_=st)
            s2 = work.tile([128, 2], F32, tag=f"s2_{t}")
            nc.vector.tensor_copy(out=s2[:, 0:1], in_=mv[:, 0:1])
            nc.vector.tensor_mul(out=s2[:, 1:2], in0=mv[:, 0:1], in1=mv[:, 0:1])
            nc.vector.tensor_add(out=s2[:, 1:2], in0=s2[:, 1:2], in1=mv[:, 1:2])
            if DEBUG_TAPS and t == 0:
                nc.sync.dma_start(out=dbg["s2"][:, :], in_=s2)
            stats2s.append(s2)

        gps = psA.tile([128, 512], F32, tag="mm")
        for t in range(4):
            nc.tensor.matmul(out=gps[0:G, 0:2], lhsT=gmap[t], rhs=stats2s[t],
                             start=(t == 0), stop=(t == 3))
        if DEBUG_TAPS:
            gpsc = work.tile([G, 2], F32, tag="gpsc")
            nc.vector.tensor_copy(out=gpsc, in_=gps[0:G, 0:2])
            nc.sync.dma_start(out=dbg["gps"][:, :], in_=gpsc)
        mvg = work.tile([G, 2], F32, tag="mvg")
        nc.vector.tensor_scalar(out=mvg, in0=gps[0:G, 0:2], scalar1=1.0 / 16,
                                scalar2=None, op0=ALU.mult)
        varg = work.tile([G, 1], F32, tag="varg")
        nc.vector.tensor_mul(out=varg, in0=mvg[:, 0:1], in1=mvg[:, 0:1])
        nc.vector.tensor_tensor(out=varg, in0=mvg[:, 1:2], in1=varg, op=ALU.subtract)
        sd = work.tile([G, 1], F32, tag="sd")
        nc.scalar.activation(out=sd, in_=varg, func=AF.Sqrt, bias=eps32)
        if DEBUG_TAPS:
            nc.sync.dma_start(out=dbg["varg"][:, :], in_=varg)
            nc.sync.dma_start(out=dbg["mvg"][:, :], in_=mvg)
        rsg = work.tile([G, 1], F32, tag="rsg")
        nc.vector.reciprocal(out=rsg, in_=sd)
        gvals = work.tile([G, 2], F32, tag="gvals")
        nc.vector.tensor_copy(out=gvals[:, 0:1], in_=rsg)
        nc.vector.tensor_copy(out=gvals[:, 1:2], in_=mvg[:, 0:1])

        hs = []
        for t in range(4):
            bc = psA.tile([128, 512], F32, tag="mm")
            nc.tensor.matmul(out=bc[:, 0:2], lhsT=gmapt[:, t * 128:(t + 1) * 128],
                             rhs=gvals, start=True, stop=True)
            a_t = work.tile([128, 1], F32, tag="a_t")
            nc.vector.tensor_mul(out=a_t, in0=bc[:, 0:1], in1=gam[t])
            b_t = work.tile([128, 1], F32, tag="b_t")
            nc.vector.tensor_mul(out=b_t, in0=bc[:, 1:2], in1=a_t)
            nc.vector.tensor_tensor(out=b_t, in0=bet[t], in1=b_t, op=ALU.subtract)
            ht = data.tile([128, N], BF16, tag=f"h{t}")
            nc.vector.tensor_scalar(out=ht, in0=xs[t], scalar1=a_t, scalar2=b_t,
                                    op0=ALU.mult, op1=ALU.add)
            if DEBUG_TAPS and t == 0:
                nc.sync.dma_start(out=dbg["h0"][:, :], in_=ht)
            hs.append(ht)

        # ---- q, k projections ----
        qs = []
        for t in range(4):
            qt = data.tile([128, NQ], BF16, tag=f"q{t}")
            for n in range(2):
                ps = psA.tile([128, 512], F32, tag="mm")
                for kt in range(4):
                    nc.tensor.matmul(out=ps, lhsT=qwt[kt][:, t * 128:(t + 1) * 128],
                                     rhs=hs[kt][:, n * 512:(n + 1) * 512],
                                     start=(kt == 0), stop=(kt == 3))
                nc.vector.tensor_scalar(out=qt[:, n * 512:(n + 1) * 512], in0=ps,
                                        scalar1=qb[t], scalar2=None, op0=ALU.add)
            if DEBUG_TAPS and t == 0:
                nc.sync.dma_start(out=dbg["q0"][:, :], in_=qt)
            qs.append(qt)
        ks = []
        for t in range(4):
            ktile = data.tile([128, N], BF16, tag=f"k{t}")
            for n in range(4):
                ps = psA.tile([128, 512], F32, tag="mm")
                for kt in range(4):
                    nc.tensor.matmul(out=ps, lhsT=kwt[kt][:, t * 128:(t + 1) * 128],
                                     rhs=hs[kt][:, n * 512:(n + 1) * 512],
                                     start=(kt == 0), stop=(kt == 3))
                nc.vector.tensor_scalar(out=ktile[:, n * 512:(n + 1) * 512], in0=ps,
                                        scalar1=kb[t], scalar2=None, op0=ALU.add)
            if DEBUG_TAPS and t == 0:
                nc.sync.dma_start(out=dbg["k0"][:, :], in_=ktile)
            ks.append(ktile)

        # ---- vT (transposed v) with ones column per head: [128, 8*65] ----
        vts = []
        for mt in range(16):
            vt = data.tile([128, H * 65], BF16, tag=f"vt{mt}")
            nc.vector.memset(vt, 1.0)
            ps = psA.tile([128, 512], F32, tag="mm")
            for kt in range(4):
                nc.tensor.matmul(out=ps, lhsT=hs[kt][:, mt * 128:(mt + 1) * 128],
                                 rhs=vwt[kt][:, 0:512], start=(kt == 0), stop=(kt == 3))
            nc.vector.tensor_copy(
                out=vt.rearrange("p (h w) -> p h w", h=H)[:, :, 0:HC],
                in_=ps.rearrange("p (h w) -> p h w", h=H),
            )
            if DEBUG_TAPS and mt == 0:
                nc.sync.dma_start(out=dbg["vt0"][:, :], in_=vt)
            vts.append(vt)

        # ---- attention core, one head-pair (= one c-tile) at a time ----
        attns = []
        for ct in range(4):
            at = data.tile([128, NQ], BF16, tag=f"attn{ct}")
            attns.append(at)

        for ct in range(4):
            ops = [[psA.tile([128, 512], F32, tag="mm", name=f"op_{ct}_{hp}_{n}")
                    for n in range(2)] for hp in range(2)]
            for mt in range(16):
                sc = psB.tile([128, N], F32, tag="sc")
                for hp in range(2):
                    hb = hp * 64
                    for n in range(2):
                        nc.tensor.matmul(
                            out=sc[:, (hp * 2 + n) * 512:(hp * 2 + n + 1) * 512],
                            lhsT=ks[ct][hb:hb + 64, mt * 128:(mt + 1) * 128],
                            rhs=qs[ct][hb:hb + 64, n * 512:(n + 1) * 512],
                            start=True, stop=True)
                et = expp.tile([128, N], BF16, tag="exp")
                nc.scalar.activation(out=et, in_=sc, func=AF.Exp, scale=float(SCALE))
                if DEBUG_TAPS and ct == 0 and mt == 0:
                    nc.sync.dma_start(out=dbg["exp0"][:, :], in_=et)
                for hp in range(2):
                    h = 2 * ct + hp
                    for n in range(2):
                        nc.tensor.matmul(
                            out=ops[hp][n][0:65, :],
                            lhsT=vts[mt][:, 65 * h:65 * h + 65],
                            rhs=et[:, (hp * 2 + n) * 512:(hp * 2 + n + 1) * 512],
                            start=(mt == 0), stop=(mt == 15))
            if DEBUG_TAPS and ct == 0:
                opc = work.tile([65, 512], F32, tag="opc")
                nc.vector.tensor_copy(out=opc, in_=ops[0][0][0:65, :])
                nc.sync.dma_start(out=dbg["op0"][:, :], in_=opc)
            for hp in range(2):
                h = 2 * ct + hp
                rec = work.tile([1, NQ], F32, tag="rec")
                for n in range(2):
                    nc.vector.reciprocal(out=rec[0:1, n * 512:(n + 1) * 512],
                                         in_=ops[hp][n][64:65, 0:512])
                recd = dpool.tile([1, NQ], F32, tag="recd")
                nc.sync.dma_start(out=recd, in_=rec)
                recb = work.tile([64, NQ], F32, tag="recb")
                nc.gpsimd.dma_start(out=recb, in_=recd.to_broadcast([64, NQ]))
                tmp = work.tile([64, NQ], BF16, tag="tmp64")
                for n in range(2):
                    nc.vector.tensor_mul(out=tmp[:, n * 512:(n + 1) * 512],
                                         in0=ops[hp][n][0:64, :],
                                         in1=recb[:, n * 512:(n + 1) * 512])
                nc.vector.tensor_scalar(out=tmp, in0=tmp, scalar1=vbh[:, h:h + 1],
                                        scalar2=None, op0=ALU.add)
                nc.sync.dma_start(out=attns[ct][hp * 64:hp * 64 + 64, :], in_=tmp)

        if DEBUG_TAPS:
            nc.sync.dma_start(out=dbg["attn0"][:, :], in_=attns[0])

        # ---- proj + bias + residual ----
        for mo in range(4):
            for n in range(2):
                ps = psA.tile([128, 512], F32, tag="mm")
                for kt in range(4):
                    nc.tensor.matmul(out=ps, lhsT=pwt[kt][:, mo * 128:(mo + 1) * 128],
                                     rhs=attns[kt][:, n * 512:(n + 1) * 512],
                                     start=(kt == 0), stop=(kt == 3))
                osb = osbp.tile([128, 512], F32, tag="osb")
                nc.vector.tensor_scalar(out=osb, in0=ps, scalar1=pb[mo],
                                        scalar2=None, op0=ALU.add)
                nc.vector.tensor_add(out=osb, in0=osb,
                                     in1=xs[mo][:, n * 512:(n + 1) * 512])
                nc.sync.dma_start(
                    out=out_d[mo * 128:(mo + 1) * 128, n * 512:(n + 1) * 512],
                    in_=osb)

    nc.compile()
    return nc


_NC_CACHE = None


def _get_nc():
    global _NC_CACHE
    if _NC_CACHE is None:
        _NC_CACHE = build_bacc()
    return _NC_CACHE


def kernel(x, gn_gamma, gn_beta, q_w, q_b, k_w, k_b, v_w, v_b, proj_w, proj_b):
    global LAST_RESULT
    x = np.asarray(x, np.float32)
    B = x.shape[0]
    bf = ml_dtypes.bfloat16

    gmap = np.zeros((C, G), np.float32)
    gmap[np.arange(C), np.arange(C) // 16] = 1.0

    shared = {
        "qwt": np.ascontiguousarray(np.asarray(q_w, np.float32).T.astype(bf)),
        "kwt": np.ascontiguousarray(np.asarray(k_w, np.float32).T.astype(bf)),
        "vwt": np.ascontiguousarray(np.asarray(v_w, np.float32).T.astype(bf)),
        "pwt": np.ascontiguousarray(np.asarray(proj_w, np.float32).T.astype(bf)),
        "qb": np.asarray(q_b, np.float32).reshape(C, 1),
        "kb": np.asarray(k_b, np.float32).reshape(C, 1),
        "pb": np.asarray(proj_b, np.float32).reshape(C, 1),
        "vbh": np.ascontiguousarray(np.asarray(v_b, np.float32).reshape(H, HC).T),
        "gam": np.asarray(gn_gamma, np.float32).reshape(C, 1),
        "bet": np.asarray(gn_beta, np.float32).reshape(C, 1),
        "gmap": gmap,
        "gmapt": np.ascontiguousarray(gmap.T),
    }

    in_maps = []
    for i in range(8):
        b, half = i // 2, i % 2
        xb = np.ascontiguousarray(np.roll(x[b], -half * NQ, axis=1))
        in_maps.append({"x": xb, **shared})

    nc = _get_nc()
    res = run_bass_kernel_spmd(nc, in_maps, core_ids=list(range(8)), trace=TRACE)
    LAST_RESULT = res

    out = np.empty((B, C, N), np.float32)
    for i in range(8):
        b, half = i // 2, i % 2
        out[b][:, half * NQ:(half + 1) * NQ] = res.results[i]["out"]
    return out


# revision 15
# speedup vs baseline: 1.4134x; 1.4134x over previous
"""AttnBlock1d Trainium2 kernel: 8-core SPMD, zero-collective sharding.

Sharding: core i handles (batch b = i//2, N-half = i%2). The input x[b] is
host-rolled along N so every core's query half sits at columns 0:1024 —
groupnorm stats, k/v (pointwise in N) and softmax are permutation-invariant
along N, so rolling commutes with everything except the q slice, which is
exactly the point.

Per-core pipeline (N=2048 keys, NQ=1024 queries, C=512, H=8 heads):
  groupnorm(x) -> h_ (bf16)
  q = qw@h_[:, :1024]+qb, k = kw@h_+kb (bf16)
  vT[n, c] = h_^T @ v_w^T (bf16, per-head 65-col blocks with a ones column)
  per head: scoresT[nk, nq] = k_h^T q_h (PSUM f32, head pairs row-packed on
  the PE array), exp via ScalarE (scale folded), out_u = vT_aug^T @ exp
  (M=65: row 64 accumulates the softmax denominator D), divide by D
  (DVE reciprocal + DMA partition-broadcast), + v_b
  proj + proj_b + residual x -> out[512, 1024]
"""

import os
import sys

import numpy as np

if "/opt/trn_rl_repo" not in sys.path:
    sys.path.insert(0, "/opt/trn_rl_repo")

import ml_dtypes

import concourse.bacc as bacc
import concourse.tile as tile
from concourse import mybir
from concourse.bass_utils import run_bass_kernel_spmd

F32 = mybir.dt.float32
BF16 = mybir.dt.bfloat16
AF = mybir.ActivationFunctionType
ALU = mybir.AluOpType

C = 512
N = 2048
NQ = 1024
H = 8
HC = 64
G = 32
EPS = 1e-6
SCALE = 1.0 / np.sqrt(C)

TRACE = False
LAST_RESULT = None


DEBUG_TAPS = False


def build_bacc():
    nc = bacc.Bacc()
    dbg = {}
    if DEBUG_TAPS:
        dbg["h0"] = nc.declare_dram_parameter("dbg_h0", [128, N], BF16, isOutput=True)
        dbg["mvg"] = nc.declare_dram_parameter("dbg_mvg", [G, 2], F32, isOutput=True)
        dbg["varg"] = nc.declare_dram_parameter("dbg_varg", [G, 1], F32, isOutput=True)
        dbg["q0"] = nc.declare_dram_parameter("dbg_q0", [128, NQ], BF16, isOutput=True)
        dbg["k0"] = nc.declare_dram_parameter("dbg_k0", [128, N], BF16, isOutput=True)
        dbg["vt0"] = nc.declare_dram_parameter("dbg_vt0", [128, H * 65], BF16, isOutput=True)
        dbg["exp0"] = nc.declare_dram_parameter("dbg_exp0", [128, NQ], BF16, isOutput=True)
        dbg["op0"] = nc.declare_dram_parameter("dbg_op0", [65, 512], F32, isOutput=True)
        dbg["s2"] = nc.declare_dram_parameter("dbg_s2", [128, 2], F32, isOutput=True)
        dbg["gps"] = nc.declare_dram_parameter("dbg_gps", [G, 2], F32, isOutput=True)
        dbg["attn0"] = nc.declare_dram_parameter("dbg_attn0", [128, NQ], BF16, isOutput=True)

    x_d = nc.declare_dram_parameter("x", [C, N], F32, isOutput=False)
    qwt_d = nc.declare_dram_parameter("qwt", [C, C], BF16, isOutput=False)
    kwt_d = nc.declare_dram_parameter("kwt", [C, C], BF16, isOutput=False)
    vwt_d = nc.declare_dram_parameter("vwt", [C, C], BF16, isOutput=False)
    pwt_d = nc.declare_dram_parameter("pwt", [C, C], BF16, isOutput=False)
    qb_d = nc.declare_dram_parameter("qb", [C, 1], F32, isOutput=False)
    kb_d = nc.declare_dram_parameter("kb", [C, 1], F32, isOutput=False)
    pb_d = nc.declare_dram_parameter("pb", [C, 1], F32, isOutput=False)
    vbh_d = nc.declare_dram_parameter("vbh", [HC, H], F32, isOutput=False)
    gam_d = nc.declare_dram_parameter("gam", [C, 1], F32, isOutput=False)
    bet_d = nc.declare_dram_parameter("bet", [C, 1], F32, isOutput=False)
    gmap_d = nc.declare_dram_parameter("gmap", [C, G], F32, isOutput=False)
    gmapt_d = nc.declare_dram_parameter("gmapt", [G, C], F32, isOutput=False)
    out_d = nc.declare_dram_parameter("out", [C, NQ], F32, isOutput=True)

    from contextlib import ExitStack

    with tile.TileContext(nc) as tc, ExitStack() as es:
        const = es.enter_context(tc.tile_pool(name="const", bufs=1))
        data = es.enter_context(tc.tile_pool(name="data", bufs=1))
        work = es.enter_context(tc.tile_pool(name="work", bufs=2))
        expp = es.enter_context(tc.tile_pool(name="expp", bufs=24))
        osbp = es.enter_context(tc.tile_pool(name="osbp", bufs=3))
        psA = es.enter_context(tc.tile_pool(name="psA", bufs=4, space="PSUM"))
        psB = es.enter_context(tc.tile_pool(name="psB", bufs=2, space="PSUM"))
        dpool = es.enter_context(tc.tile_pool(name="dpool", bufs=2, space="DRAM"))

        xs = []
        for t in range(4):
            xt = data.tile([128, N], F32, tag=f"x{t}")
            nc.sync.dma_start(out=xt, in_=x_d[t * 128:(t + 1) * 128, :])
            xs.append(xt)

        # ---- constant loads ----
        def load4(dram, shape, dt, tagp):
            ts = []
            for t in range(4):
                s = const.tile(shape, dt, tag=f"{tagp}{t}")
                nc.sync.dma_start(out=s, in_=dram[t * 128:(t + 1) * 128, :])
                ts.append(s)
            return ts

        qwt = load4(qwt_d, [128, C], BF16, "qwt")
        kwt = load4(kwt_d, [128, C], BF16, "kwt")
        vwt = load4(vwt_d, [128, C], BF16, "vwt")
        pwt = load4(pwt_d, [128, C], BF16, "pwt")
        qb = load4(qb_d, [128, 1], F32, "qb")
        kb = load4(kb_d, [128, 1], F32, "kb")
        pb = load4(pb_d, [128, 1], F32, "pb")
        gam = load4(gam_d, [128, 1], F32, "gam")
        bet = load4(bet_d, [128, 1], F32, "bet")
        gmap = load4(gmap_d, [128, G], F32, "gmap")
        gmapt = const.tile([G, C], F32, tag="gmapt")
        nc.sync.dma_start(out=gmapt, in_=gmapt_d[:, :])
        vbh = const.tile([HC, H], F32, tag="vbh")
        nc.sync.dma_start(out=vbh, in_=vbh_d[:, :])
        eps32 = const.tile([G, 1], F32, tag="eps32")
        nc.vector.memset(eps32, EPS)

        # ---- groupnorm stats ----
        stats2s = []
        for t in range(4):
            st = work.tile([128, 4, 6], F32, tag="bnst")
            for sg in range(4):
                nc.vector.bn_stats(out=st[:, sg, :], in_=xs[t][:, sg * 512:(sg + 1) * 512])
            mv = work.tile([128, 2], F32, tag="bnmv")
            nc.vector.bn_aggr(out=mv, in_=st)
            s2 = work.tile([128, 2], F32, tag=f"s2_{t}")
            nc.vector.tensor_copy(out=s2[:, 0:1], in_=mv[:, 0:1])
            nc.vector.tensor_mul(out=s2[:, 1:2], in0=mv[:, 0:1], in1=mv[:, 0:1])
            nc.vector.tensor_add(out=s2[:, 1:2], in0=s2[:, 1:2], in1=mv[:, 1:2])
            if DEBUG_TAPS and t == 0:
                nc.sync.dma_start(out=dbg["s2"][:, :], in_=s2)
            stats2s.append(s2)

        gps = psA.tile([128, 512], F32, tag="mm")
        for t in range(4):
            nc.tensor.matmul(out=gps[0:G, 0:2], lhsT=gmap[t], rhs=stats2s[t],
                             start=(t == 0), stop=(t == 3))
        if DEBUG_TAPS:
            gpsc = work.tile([G, 2], F32, tag="gpsc")
            nc.vector.tensor_copy(out=gpsc, in_=gps[0:G, 0:2])
            nc.sync.dma_start(out=dbg["gps"][:, :], in_=gpsc)
        mvg = work.tile([G, 2], F32, tag="mvg")
        nc.vector.tensor_scalar(out=mvg, in0=gps[0:G, 0:2], scalar1=1.0 / 16,
                                scalar2=None, op0=ALU.mult)
        varg = work.tile([G, 1], F32, tag="varg")
        nc.vector.tensor_mul(out=varg, in0=mvg[:, 0:1], in1=mvg[:, 0:1])
        nc.vector.tensor_tensor(out=varg, in0=mvg[:, 1:2], in1=varg, op=ALU.subtract)
        sd = work.tile([G, 1], F32, tag="sd")
        nc.scalar.activation(out=sd, in_=varg, func=AF.Sqrt, bias=eps32)
        if DEBUG_TAPS:
            nc.sync.dma_start(out=dbg["varg"][:, :], in_=varg)
            nc.sync.dma_start(out=dbg["mvg"][:, :], in_=mvg)
        rsg = work.tile([G, 1], F32, tag="rsg")
        nc.vector.reciprocal(out=rsg, in_=sd)
        gvals = work.tile([G, 2], F32, tag="gvals")
        nc.vector.tensor_copy(out=gvals[:, 0:1], in_=rsg)
        nc.vector.tensor_copy(out=gvals[:, 1:2], in_=mvg[:, 0:1])

        hs = []
        for t in range(4):
            bc = psA.tile([128, 512], F32, tag="mm")
            nc.tensor.matmul(out=bc[:, 0:2], lhsT=gmapt[:, t * 128:(t + 1) * 128],
                             rhs=gvals, start=True, stop=True)
            a_t = work.tile([128, 1], F32, tag="a_t")
            nc.vector.tensor_mul(out=a_t, in0=bc[:, 0:1], in1=gam[t])
            b_t = work.tile([128, 1], F32, tag="b_t")
            nc.vector.tensor_mul(out=b_t, in0=bc[:, 1:2], in1=a_t)
            nc.vector.tensor_tensor(out=b_t, in0=bet[t], in1=b_t, op=ALU.subtract)
            ht = data.tile([128, N], BF16, tag=f"h{t}")
            nc.vector.tensor_scalar(out=ht, in0=xs[t], scalar1=a_t, scalar2=b_t,
                                    op0=ALU.mult, op1=ALU.add)
            if DEBUG_TAPS and t == 0:
                nc.sync.dma_start(out=dbg["h0"][:, :], in_=ht)
            hs.append(ht)

        # ---- q, k projections ----
        qs = []
        for t in range(4):
            qt = data.tile([128, NQ], BF16, tag=f"q{t}")
            for n in range(2):
                ps = psA.tile([128, 512], F32, tag="mm")
                for kt in range(4):
                    nc.tensor.matmul(out=ps, lhsT=qwt[kt][:, t * 128:(t + 1) * 128],
                                     rhs=hs[kt][:, n * 512:(n + 1) * 512],
                                     start=(kt == 0), stop=(kt == 3))
                nc.vector.tensor_scalar(out=qt[:, n * 512:(n + 1) * 512], in0=ps,
                                        scalar1=qb[t], scalar2=None, op0=ALU.add)
            if DEBUG_TAPS and t == 0:
                nc.sync.dma_start(out=dbg["q0"][:, :], in_=qt)
            qs.append(qt)
        ks = []
        for t in range(4):
            ktile = data.tile([128, N], BF16, tag=f"k{t}")
            for n in range(4):
                ps = psA.tile([128, 512], F32, tag="mm")
                for kt in range(4):
                    nc.tensor.matmul(out=ps, lhsT=kwt[kt][:, t * 128:(t + 1) * 128],
                                     rhs=hs[kt][:, n * 512:(n + 1) * 512],
                                     start=(kt == 0), stop=(kt == 3))
                nc.vector.tensor_scalar(out=ktile[:, n * 512:(n + 1) * 512], in0=ps,
                                        scalar1=kb[t], scalar2=None, op0=ALU.add)
            if DEBUG_TAPS and t == 0:
                nc.sync.dma_start(out=dbg["k0"][:, :], in_=ktile)
            ks.append(ktile)

        # ---- vT (transposed v) with ones column per head: [128, 8*65] ----
        vts = []
        for mt in range(16):
            vt = data.tile([128, H * 65], BF16, tag=f"vt{mt}")
            nc.vector.memset(vt, 1.0)
            ps = psA.tile([128, 512], F32, tag="mm")
            for kt in range(4):
                nc.tensor.matmul(out=ps, lhsT=hs[kt][:, mt * 128:(mt + 1) * 128],
                                 rhs=vwt[kt][:, 0:512], start=(kt == 0), stop=(kt == 3))
            nc.vector.tensor_copy(
                out=vt.rearrange("p (h w) -> p h w", h=H)[:, :, 0:HC],
                in_=ps.rearrange("p (h w) -> p h w", h=H),
            )
            if DEBUG_TAPS and mt == 0:
                nc.sync.dma_start(out=dbg["vt0"][:, :], in_=vt)
            vts.append(vt)

        # ---- attention core, one head-pair (= one c-tile) at a time ----
        attns = []
        for ct in range(4):
            at = data.tile([128, NQ], BF16, tag=f"attn{ct}")
            attns.append(at)

        for ct in range(4):
            ops = [[psA.tile([128, 512], F32, tag="mm", name=f"op_{ct}_{hp}_{n}")
                    for n in range(2)] for hp in range(2)]
            for mt in range(16):
                for n in range(2):
                    sc = psB.tile([128, NQ], F32, tag="sc", name=f"sc_{ct}_{mt}_{n}")
                    for hp in range(2):
                        hb = hp * 64
                        nc.tensor.matmul(
                            out=sc[:, hp * 512:(hp + 1) * 512],
                            lhsT=ks[ct][hb:hb + 64, mt * 128:(mt + 1) * 128],
                            rhs=qs[ct][hb:hb + 64, n * 512:(n + 1) * 512],
                            start=True, stop=True)
                    et = expp.tile([128, NQ], BF16, tag="exp", name=f"et_{ct}_{mt}_{n}")
                    nc.scalar.activation(out=et, in_=sc, func=AF.Exp, scale=float(SCALE))
                    if DEBUG_TAPS and ct == 0 and mt == 0 and n == 0:
                        nc.sync.dma_start(out=dbg["exp0"][:, 0:512], in_=et[:, 0:512])
                        nc.sync.dma_start(out=dbg["exp0"][:, 512:1024], in_=et[:, 512:1024])
                    for hp in range(2):
                        h = 2 * ct + hp
                        nc.tensor.matmul(
                            out=ops[hp][n][0:65, :],
                            lhsT=vts[mt][:, 65 * h:65 * h + 65],
                            rhs=et[:, hp * 512:(hp + 1) * 512],
                            start=(mt == 0), stop=(mt == 15))
            if DEBUG_TAPS and ct == 0:
                opc = work.tile([65, 512], F32, tag="opc")
                nc.vector.tensor_copy(out=opc, in_=ops[0][0][0:65, :])
                nc.sync.dma_start(out=dbg["op0"][:, :], in_=opc)
            rec = work.tile([1, 4 * 512], F32, tag="rec")
            for hp in range(2):
                for n in range(2):
                    nc.vector.tensor_copy(
                        out=rec[0:1, (hp * 2 + n) * 512:(hp * 2 + n + 1) * 512],
                        in_=ops[hp][n][64:65, 0:512])
            recd1 = dpool.tile([1, 4 * 512], F32, tag="recd1")
            nc.sync.dma_start(out=recd1, in_=rec)
            rct = work.tile([128, 16], F32, tag="rct")
            nc.sync.dma_start(out=rct,
                              in_=recd1.rearrange("o (f p) -> (o p) f", p=128))
            nc.vector.reciprocal(out=rct, in_=rct)
            recd2 = dpool.tile([1, 4 * 512], F32, tag="recd2")
            nc.sync.dma_start(out=recd2.rearrange("o (f p) -> (o p) f", p=128),
                              in_=rct)
            for hp in range(2):
                h = 2 * ct + hp
                recb = work.tile([64, NQ], F32, tag="recb")
                for n in range(2):
                    nc.gpsimd.dma_start(
                        out=recb[:, n * 512:(n + 1) * 512],
                        in_=recd2[0:1, (hp * 2 + n) * 512:(hp * 2 + n + 1) * 512]
                        .to_broadcast([64, 512]))
                tmp = work.tile([64, NQ], BF16, tag="tmp64")
                for n in range(2):
                    nc.vector.tensor_mul(out=tmp[:, n * 512:(n + 1) * 512],
                                         in0=ops[hp][n][0:64, :],
                                         in1=recb[:, n * 512:(n + 1) * 512])
                nc.vector.tensor_scalar(out=tmp, in0=tmp, scalar1=vbh[:, h:h + 1],
                                        scalar2=None, op0=ALU.add)
                nc.sync.dma_start(out=attns[ct][hp * 64:hp * 64 + 64, :], in_=tmp)

        if DEBUG_TAPS:
            nc.sync.dma_start(out=dbg["attn0"][:, :], in_=attns[0])

        # ---- proj + bias + residual ----
        for mo in range(4):
            for n in range(2):
                ps = psA.tile([128, 512], F32, tag="mm")
                for kt in range(4):
                    nc.tensor.matmul(out=ps, lhsT=pwt[kt][:, mo * 128:(mo + 1) * 128],
                                     rhs=attns[kt][:, n * 512:(n + 1) * 512],
                                     start=(kt == 0), stop=(kt == 3))
                osb = osbp.tile([128, 512], F32, tag="osb")
                nc.vector.tensor_scalar(out=osb, in0=ps, scalar1=pb[mo],
                                        scalar2=None, op0=ALU.add)
                nc.vector.tensor_add(out=osb, in0=osb,
                                     in1=xs[mo][:, n * 512:(n + 1) * 512])
                nc.sync.dma_start(
                    out=out_d[mo * 128:(mo + 1) * 128, n * 512:(n + 1) * 512],
                    in_=osb)

    nc.compile()
    return nc


_NC_CACHE = None


def _get_nc():
    global _NC_CACHE
    if _NC_CACHE is None:
        _NC_CACHE = build_bacc()
    return _NC_CACHE


def kernel(x, gn_gamma, gn_beta, q_w, q_b, k_w, k_b, v_w, v_b, proj_w, proj_b):
    global LAST_RESULT
    x = np.asarray(x, np.float32)
    B = x.shape[0]
    bf = ml_dtypes.bfloat16

    gmap = np.zeros((C, G), np.float32)
    gmap[np.arange(C), np.arange(C) // 16] = 1.0

    shared = {
        "qwt": np.ascontiguousarray(np.asarray(q_w, np.float32).T.astype(bf)),
        "kwt": np.ascontiguousarray(np.asarray(k_w, np.float32).T.astype(bf)),
        "vwt": np.ascontiguousarray(np.asarray(v_w, np.float32).T.astype(bf)),
        "pwt": np.ascontiguousarray(np.asarray(proj_w, np.float32).T.astype(bf)),
        "qb": np.asarray(q_b, np.float32).reshape(C, 1),
        "kb": np.asarray(k_b, np.float32).reshape(C, 1),
        "pb": np.asarray(proj_b, np.float32).reshape(C, 1),
        "vbh": np.ascontiguousarray(np.asarray(v_b, np.float32).reshape(H, HC).T),
        "gam": np.asarray(gn_gamma, np.float32).reshape(C, 1),
        "bet": np.asarray(gn_beta, np.float32).reshape(C, 1),
        "gmap": gmap,
        "gmapt": np.ascontiguousarray(gmap.T),
    }

    in_maps = []
    for i in range(8):
        b, half = i // 2, i % 2
        xb = np.ascontiguousarray(np.roll(x[b], -half * NQ, axis=1))
        in_maps.append({"x": xb, **shared})

    nc = _get_nc()
    res = run_bass_kernel_spmd(nc, in_maps, core_ids=list(range(8)), trace=TRACE)
    LAST_RESULT = res

    out = np.empty((B, C, N), np.float32)
    for i in range(8):
        b, half = i // 2, i % 2
        out[b][:, half * NQ:(half + 1) * NQ] = res.results[i]["out"]
    return out


# revision 18
# speedup vs baseline: 1.6892x; 1.1951x over previous
"""AttnBlock1d Trainium2 kernel: 8-core SPMD, zero-collective sharding.

Sharding: core i handles (batch b = i//2, N-half = i%2). The input x[b] is
host-rolled along N so every core's query half sits at columns 0:1024 —
groupnorm stats, k/v (pointwise in N) and softmax are permutation-invariant
along N, so rolling commutes with everything except the q slice, which is
exactly the point.

Per-core pipeline (N=2048 keys, NQ=1024 queries, C=512, H=8 heads):
  groupnorm(x) -> h_ (bf16)
  q = qw@h_[:, :1024]+qb, k = kw@h_+kb (bf16)
  vT[n, c] = h_^T @ v_w^T (bf16, per-head 65-col blocks with a ones column)
  per head: scoresT[nk, nq] = k_h^T q_h (PSUM f32, head pairs row-packed on
  the PE array), exp via ScalarE (scale folded), out_u = vT_aug^T @ exp
  (M=65: row 64 accumulates the softmax denominator D), divide by D
  (DVE reciprocal + DMA partition-broadcast), + v_b
  proj + proj_b + residual x -> out[512, 1024]
"""

import os
import sys

import numpy as np

if "/opt/trn_rl_repo" not in sys.path:
    sys.path.insert(0, "/opt/trn_rl_repo")

import ml_dtypes

import concourse.bacc as bacc
import concourse.tile as tile
from concourse import mybir
from concourse.bass_utils import run_bass_kernel_spmd

F32 = mybir.dt.float32
BF16 = mybir.dt.bfloat16
AF = mybir.ActivationFunctionType
ALU = mybir.AluOpType

C = 512
N = 2048
NQ = 1024
H = 8
HC = 64
G = 32
EPS = 1e-6
SCALE = 1.0 / np.sqrt(C)

TRACE = False
LAST_RESULT = None


DEBUG_TAPS = False


def build_bacc():
    nc = bacc.Bacc()
    dbg = {}
    if DEBUG_TAPS:
        dbg["h0"] = nc.declare_dram_parameter("dbg_h0", [128, N], BF16, isOutput=True)
        dbg["mvg"] = nc.declare_dram_parameter("dbg_mvg", [G, 2], F32, isOutput=True)
        dbg["varg"] = nc.declare_dram_parameter("dbg_varg", [G, 1], F32, isOutput=True)
        dbg["q0"] = nc.declare_dram_parameter("dbg_q0", [128, NQ], BF16, isOutput=True)
        dbg["k0"] = nc.declare_dram_parameter("dbg_k0", [128, N], BF16, isOutput=True)
        dbg["vt0"] = nc.declare_dram_parameter("dbg_vt0", [128, H * 65], BF16, isOutput=True)
        dbg["exp0"] = nc.declare_dram_parameter("dbg_exp0", [128, NQ], BF16, isOutput=True)
        dbg["op0"] = nc.declare_dram_parameter("dbg_op0", [65, 512], F32, isOutput=True)
        dbg["s2"] = nc.declare_dram_parameter("dbg_s2", [128, 2], F32, isOutput=True)
        dbg["gps"] = nc.declare_dram_parameter("dbg_gps", [G, 2], F32, isOutput=True)
        dbg["attn0"] = nc.declare_dram_parameter("dbg_attn0", [128, NQ], BF16, isOutput=True)

    x_d = nc.declare_dram_parameter("x", [C, N], F32, isOutput=False)
    qwt_d = nc.declare_dram_parameter("qwt", [C, C], BF16, isOutput=False)
    kwt_d = nc.declare_dram_parameter("kwt", [C, C], BF16, isOutput=False)
    vwt_d = nc.declare_dram_parameter("vwt", [C, C], BF16, isOutput=False)
    pwt_d = nc.declare_dram_parameter("pwt", [C, C], BF16, isOutput=False)
    qb_d = nc.declare_dram_parameter("qb", [C, 1], F32, isOutput=False)
    kb_d = nc.declare_dram_parameter("kb", [C, 1], F32, isOutput=False)
    pb_d = nc.declare_dram_parameter("pb", [C, 1], F32, isOutput=False)
    vbh_d = nc.declare_dram_parameter("vbh", [HC, H], F32, isOutput=False)
    gam_d = nc.declare_dram_parameter("gam", [C, 1], F32, isOutput=False)
    bet_d = nc.declare_dram_parameter("bet", [C, 1], F32, isOutput=False)
    gmap_d = nc.declare_dram_parameter("gmap", [C, G], F32, isOutput=False)
    gmapt_d = nc.declare_dram_parameter("gmapt", [G, C], F32, isOutput=False)
    out_d = nc.declare_dram_parameter("out", [C, NQ], F32, isOutput=True)

    from contextlib import ExitStack

    with tile.TileContext(nc) as tc, ExitStack() as es:
        const = es.enter_context(tc.tile_pool(name="const", bufs=1))
        data = es.enter_context(tc.tile_pool(name="data", bufs=1))
        work = es.enter_context(tc.tile_pool(name="work", bufs=2))
        expp = es.enter_context(tc.tile_pool(name="expp", bufs=24))
        osbp = es.enter_context(tc.tile_pool(name="osbp", bufs=3))
        psA = es.enter_context(tc.tile_pool(name="psA", bufs=4, space="PSUM"))
        psB = es.enter_context(tc.tile_pool(name="psB", bufs=2, space="PSUM"))
        dpool = es.enter_context(tc.tile_pool(name="dpool", bufs=2, space="DRAM"))

        xs = []
        xdma = [nc.sync, nc.scalar]
        for t in range(4):
            xt = data.tile([128, N], F32, tag=f"x{t}")
            eng = xdma[t % len(xdma)]
            eng.dma_start(out=xt[:, 0:1024], in_=x_d[t * 128:(t + 1) * 128, 0:1024])
            eng.dma_start(out=xt[:, 1024:2048],
                          in_=x_d[t * 128:(t + 1) * 128, 1024:2048])
            xs.append(xt)

        # ---- constant loads ----
        wdma = [nc.gpsimd, nc.sync]
        wdma_i = [0]

        def load4(dram, shape, dt, tagp):
            ts = []
            for t in range(4):
                s = const.tile(shape, dt, tag=f"{tagp}{t}")
                eng = wdma[wdma_i[0] % len(wdma)]
                wdma_i[0] += 1
                eng.dma_start(out=s, in_=dram[t * 128:(t + 1) * 128, :])
                ts.append(s)
            return ts

        qwt = load4(qwt_d, [128, C], BF16, "qwt")
        kwt = load4(kwt_d, [128, C], BF16, "kwt")
        vwt = load4(vwt_d, [128, C], BF16, "vwt")
        pwt = load4(pwt_d, [128, C], BF16, "pwt")
        qb = load4(qb_d, [128, 1], F32, "qb")
        kb = load4(kb_d, [128, 1], F32, "kb")
        pb = load4(pb_d, [128, 1], F32, "pb")
        gam = load4(gam_d, [128, 1], F32, "gam")
        bet = load4(bet_d, [128, 1], F32, "bet")
        gmap = load4(gmap_d, [128, G], F32, "gmap")
        gmapt = const.tile([G, C], F32, tag="gmapt")
        nc.sync.dma_start(out=gmapt, in_=gmapt_d[:, :])
        vbh = const.tile([HC, H], F32, tag="vbh")
        nc.sync.dma_start(out=vbh, in_=vbh_d[:, :])
        eps32 = const.tile([G, 1], F32, tag="eps32")
        nc.vector.memset(eps32, EPS)

        # ---- groupnorm stats ----
        stats2s = []
        for t in range(4):
            st = work.tile([128, 4, 6], F32, tag="bnst")
            for sg in range(4):
                nc.vector.bn_stats(out=st[:, sg, :], in_=xs[t][:, sg * 512:(sg + 1) * 512])
            mv = work.tile([128, 2], F32, tag="bnmv")
            nc.vector.bn_aggr(out=mv, in_=st)
            s2 = work.tile([128, 2], F32, tag=f"s2_{t}")
            nc.vector.tensor_copy(out=s2[:, 0:1], in_=mv[:, 0:1])
            nc.vector.tensor_mul(out=s2[:, 1:2], in0=mv[:, 0:1], in1=mv[:, 0:1])
            nc.vector.tensor_add(out=s2[:, 1:2], in0=s2[:, 1:2], in1=mv[:, 1:2])
            if DEBUG_TAPS and t == 0:
                nc.sync.dma_start(out=dbg["s2"][:, :], in_=s2)
            stats2s.append(s2)

        gps = psA.tile([128, 512], F32, tag="mm")
        for t in range(4):
            nc.tensor.matmul(out=gps[0:G, 0:2], lhsT=gmap[t], rhs=stats2s[t],
                             start=(t == 0), stop=(t == 3))
        if DEBUG_TAPS:
            gpsc = work.tile([G, 2], F32, tag="gpsc")
            nc.vector.tensor_copy(out=gpsc, in_=gps[0:G, 0:2])
            nc.sync.dma_start(out=dbg["gps"][:, :], in_=gpsc)
        mvg = work.tile([G, 2], F32, tag="mvg")
        nc.vector.tensor_scalar(out=mvg, in0=gps[0:G, 0:2], scalar1=1.0 / 16,
                                scalar2=None, op0=ALU.mult)
        varg = work.tile([G, 1], F32, tag="varg")
        nc.vector.tensor_mul(out=varg, in0=mvg[:, 0:1], in1=mvg[:, 0:1])
        nc.vector.tensor_tensor(out=varg, in0=mvg[:, 1:2], in1=varg, op=ALU.subtract)
        sd = work.tile([G, 1], F32, tag="sd")
        nc.scalar.activation(out=sd, in_=varg, func=AF.Sqrt, bias=eps32)
        if DEBUG_TAPS:
            nc.sync.dma_start(out=dbg["varg"][:, :], in_=varg)
            nc.sync.dma_start(out=dbg["mvg"][:, :], in_=mvg)
        rsg = work.tile([G, 1], F32, tag="rsg")
        nc.vector.reciprocal(out=rsg, in_=sd)
        gvals = work.tile([G, 2], F32, tag="gvals")
        nc.vector.tensor_copy(out=gvals[:, 0:1], in_=rsg)
        nc.vector.tensor_copy(out=gvals[:, 1:2], in_=mvg[:, 0:1])

        hs = []
        for t in range(4):
            bc = psA.tile([128, 512], F32, tag="mm")
            nc.tensor.matmul(out=bc[:, 0:2], lhsT=gmapt[:, t * 128:(t + 1) * 128],
                             rhs=gvals, start=True, stop=True)
            a_t = work.tile([128, 1], F32, tag="a_t")
            nc.vector.tensor_mul(out=a_t, in0=bc[:, 0:1], in1=gam[t])
            b_t = work.tile([128, 1], F32, tag="b_t")
            nc.vector.tensor_mul(out=b_t, in0=bc[:, 1:2], in1=a_t)
            nc.vector.tensor_tensor(out=b_t, in0=bet[t], in1=b_t, op=ALU.subtract)
            ht = data.tile([128, N], BF16, tag=f"h{t}")
            nc.vector.tensor_scalar(out=ht, in0=xs[t], scalar1=a_t, scalar2=b_t,
                                    op0=ALU.mult, op1=ALU.add)
            if DEBUG_TAPS and t == 0:
                nc.sync.dma_start(out=dbg["h0"][:, :], in_=ht)
            hs.append(ht)

        # ---- q, k projections ----
        qs = []
        for t in range(4):
            qt = data.tile([128, NQ], BF16, tag=f"q{t}")
            for n in range(2):
                ps = psA.tile([128, 512], F32, tag="mm")
                for kt in range(4):
                    nc.tensor.matmul(out=ps, lhsT=qwt[kt][:, t * 128:(t + 1) * 128],
                                     rhs=hs[kt][:, n * 512:(n + 1) * 512],
                                     start=(kt == 0), stop=(kt == 3))
                nc.vector.tensor_scalar(out=qt[:, n * 512:(n + 1) * 512], in0=ps,
                                        scalar1=qb[t], scalar2=None, op0=ALU.add)
            if DEBUG_TAPS and t == 0:
                nc.sync.dma_start(out=dbg["q0"][:, :], in_=qt)
            qs.append(qt)
        ks = []
        for t in range(4):
            ktile = data.tile([128, N], BF16, tag=f"k{t}")
            for n in range(4):
                ps = psA.tile([128, 512], F32, tag="mm")
                for kt in range(4):
                    nc.tensor.matmul(out=ps, lhsT=kwt[kt][:, t * 128:(t + 1) * 128],
                                     rhs=hs[kt][:, n * 512:(n + 1) * 512],
                                     start=(kt == 0), stop=(kt == 3))
                nc.vector.tensor_scalar(out=ktile[:, n * 512:(n + 1) * 512], in0=ps,
                                        scalar1=kb[t], scalar2=None, op0=ALU.add)
            if DEBUG_TAPS and t == 0:
                nc.sync.dma_start(out=dbg["k0"][:, :], in_=ktile)
            ks.append(ktile)

        # ---- vT (transposed v) with ones column per head: [128, 8*65] ----
        vts = []
        for mt in range(16):
            vt = data.tile([128, H * 65], BF16, tag=f"vt{mt}")
            nc.vector.memset(vt, 1.0)
            ps = psA.tile([128, 512], F32, tag="mm")
            for kt in range(4):
                nc.tensor.matmul(out=ps, lhsT=hs[kt][:, mt * 128:(mt + 1) * 128],
                                 rhs=vwt[kt][:, 0:512], start=(kt == 0), stop=(kt == 3))
            nc.vector.tensor_copy(
                out=vt.rearrange("p (h w) -> p h w", h=H)[:, :, 0:HC],
                in_=ps.rearrange("p (h w) -> p h w", h=H),
            )
            if DEBUG_TAPS and mt == 0:
                nc.sync.dma_start(out=dbg["vt0"][:, :], in_=vt)
            vts.append(vt)

        # ---- attention core, one head-pair (= one c-tile) at a time ----
        attns = []
        for ct in range(4):
            at = data.tile([128, NQ], BF16, tag=f"attn{ct}")
            attns.append(at)

        for ct in range(4):
            ops = [[psA.tile([128, 512], F32, tag="mm", name=f"op_{ct}_{hp}_{n}")
                    for n in range(2)] for hp in range(2)]
            for mt in range(16):
                for n in range(2):
                    sc = psB.tile([128, NQ], F32, tag="sc", name=f"sc_{ct}_{mt}_{n}")
                    for hp in range(2):
                        hb = hp * 64
                        nc.tensor.matmul(
                            out=sc[:, hp * 512:(hp + 1) * 512],
                            lhsT=ks[ct][hb:hb + 64, mt * 128:(mt + 1) * 128],
                            rhs=qs[ct][hb:hb + 64, n * 512:(n + 1) * 512],
                            start=True, stop=True)
                    et = expp.tile([128, NQ], BF16, tag="exp", name=f"et_{ct}_{mt}_{n}")
                    nc.scalar.activation(out=et, in_=sc, func=AF.Exp, scale=float(SCALE))
                    if DEBUG_TAPS and ct == 0 and mt == 0 and n == 0:
                        nc.sync.dma_start(out=dbg["exp0"][:, 0:512], in_=et[:, 0:512])
                        nc.sync.dma_start(out=dbg["exp0"][:, 512:1024], in_=et[:, 512:1024])
                    for hp in range(2):
                        h = 2 * ct + hp
                        nc.tensor.matmul(
                            out=ops[hp][n][0:65, :],
                            lhsT=vts[mt][:, 65 * h:65 * h + 65],
                            rhs=et[:, hp * 512:(hp + 1) * 512],
                            start=(mt == 0), stop=(mt == 15))
            if DEBUG_TAPS and ct == 0:
                opc = work.tile([65, 512], F32, tag="opc")
                nc.vector.tensor_copy(out=opc, in_=ops[0][0][0:65, :])
                nc.sync.dma_start(out=dbg["op0"][:, :], in_=opc)
            oc = work.tile([65, 4 * 512], F32, tag="oc")
            for hp in range(2):
                for n in range(2):
                    nc.vector.tensor_copy(
                        out=oc[:, (hp * 2 + n) * 512:(hp * 2 + n + 1) * 512],
                        in_=ops[hp][n][0:65, 0:512])
            recd1 = dpool.tile([1, 4 * 512], F32, tag="recd1")
            nc.sync.dma_start(out=recd1, in_=oc[64:65, :])
            rct = work.tile([128, 16], F32, tag="rct")
            nc.sync.dma_start(out=rct,
                              in_=recd1.rearrange("o (f p) -> (o p) f", p=128))
            nc.vector.reciprocal(out=rct, in_=rct)
            recd2 = dpool.tile([1, 4 * 512], F32, tag="recd2")
            nc.sync.dma_start(out=recd2.rearrange("o (f p) -> (o p) f", p=128),
                              in_=rct)
            for hp in range(2):
                h = 2 * ct + hp
                recb = work.tile([64, NQ], F32, tag="recb")
                for n in range(2):
                    nc.gpsimd.dma_start(
                        out=recb[:, n * 512:(n + 1) * 512],
                        in_=recd2[0:1, (hp * 2 + n) * 512:(hp * 2 + n + 1) * 512]
                        .to_broadcast([64, 512]))
                tmp = work.tile([64, NQ], BF16, tag="tmp64")
                for n in range(2):
                    nc.vector.tensor_mul(
                        out=tmp[:, n * 512:(n + 1) * 512],
                        in0=oc[0:64, (hp * 2 + n) * 512:(hp * 2 + n + 1) * 512],
                        in1=recb[:, n * 512:(n + 1) * 512])
                nc.vector.tensor_scalar(out=tmp, in0=tmp, scalar1=vbh[:, h:h + 1],
                                        scalar2=None, op0=ALU.add)
                nc.sync.dma_start(out=attns[ct][hp * 64:hp * 64 + 64, :], in_=tmp)

        if DEBUG_TAPS:
            nc.sync.dma_start(out=dbg["attn0"][:, :], in_=attns[0])

        # ---- proj + bias + residual ----
        for mo in range(4):
            for n in range(2):
                ps = psA.tile([128, 512], F32, tag="mm")
                for kt in range(4):
                    nc.tensor.matmul(out=ps, lhsT=pwt[kt][:, mo * 128:(mo + 1) * 128],
                                     rhs=attns[kt][:, n * 512:(n + 1) * 512],
                                     start=(kt == 0), stop=(kt == 3))
                osb = osbp.tile([128, 512], F32, tag="osb")
                nc.vector.tensor_scalar(out=osb, in0=ps, scalar1=pb[mo],
                                        scalar2=None, op0=ALU.add)
                nc.vector.tensor_add(out=osb, in0=osb,
                                     in1=xs[mo][:, n * 512:(n + 1) * 512])
                nc.sync.dma_start(
                    out=out_d[mo * 128:(mo + 1) * 128, n * 512:(n + 1) * 512],
                    in_=osb)

    nc.compile()
    return nc


_NC_CACHE = None


def _get_nc():
    global _NC_CACHE
    if _NC_CACHE is None:
        _NC_CACHE = build_bacc()
    return _NC_CACHE


def kernel(x, gn_gamma, gn_beta, q_w, q_b, k_w, k_b, v_w, v_b, proj_w, proj_b):
    global LAST_RESULT
    x = np.asarray(x, np.float32)
    B = x.shape[0]
    bf = ml_dtypes.bfloat16

    gmap = np.zeros((C, G), np.float32)
    gmap[np.arange(C), np.arange(C) // 16] = 1.0

    shared = {
        "qwt": np.ascontiguousarray(np.asarray(q_w, np.float32).T.astype(bf)),
        "kwt": np.ascontiguousarray(np.asarray(k_w, np.float32).T.astype(bf)),
        "vwt": np.ascontiguousarray(np.asarray(v_w, np.float32).T.astype(bf)),
        "pwt": np.ascontiguousarray(np.asarray(proj_w, np.float32).T.astype(bf)),
        "qb": np.asarray(q_b, np.float32).reshape(C, 1),
        "kb": np.asarray(k_b, np.float32).reshape(C, 1),
        "pb": np.asarray(proj_b, np.float32).reshape(C, 1),
        "vbh": np.ascontiguousarray(np.asarray(v_b, np.float32).reshape(H, HC).T),
        "gam": np.asarray(gn_gamma, np.float32).reshape(C, 1),
        "bet": np.asarray(gn_beta, np.float32).reshape(C, 1),
        "gmap": gmap,
        "gmapt": np.ascontiguousarray(gmap.T),
    }

    in_maps = []
    for i in range(8):
        b, half = i // 2, i % 2
        xb = np.ascontiguousarray(np.roll(x[b], -half * NQ, axis=1))
        in_maps.append({"x": xb, **shared})

    nc = _get_nc()
    res = run_bass_kernel_spmd(nc, in_maps, core_ids=list(range(8)), trace=TRACE)
    LAST_RESULT = res

    out = np.empty((B, C, N), np.float32)
    for i in range(8):
        b, half = i // 2, i % 2
        out[b][:, half * NQ:(half + 1) * NQ] = res.results[i]["out"]
    return out
